# revision 5
# baseline (speedup 1.0000x reference)
# Trainium2 Bass kernel for a 2-layer GraphSAGE encoder (SAGEConv mean aggr).
#
#   h   = relu(mean_nbr(x) @ W1_l + b1 + x @ W1_r)
#   out = mean_nbr(h) @ W2_l + b2 + h @ W2_r
#
# v3: SWDGE dma_gather (int16 lo/hi halves) with a bf16 data path, 512-dst
# PSUM windows, 4 SWDGE queues, bf16 AllGather between layers. Data-parallel
# over destination nodes (8 cores), degree-balanced snake deal.
import os
import sys
import numpy as np

for _p in ("/opt/trn_rl_repo",):
    if _p not in sys.path and os.path.isdir(_p):
        sys.path.append(_p)

import ml_dtypes
import concourse.bass as bass
import concourse.bacc as bacc
import concourse.mybir as mybir
from concourse import tile
from concourse.bass_utils import run_bass_kernel_spmd

F32 = mybir.dt.float32
BF16 = mybir.dt.bfloat16
I16 = mybir.dt.int16

N_CORES = 8
WIN = 512        # dst nodes per aggregation window (one PSUM bank)
CT = 8           # gather chunk size in 128-slot tiles (ucode caps 1024 idxs)
NBATCH = 512     # dense-phase node batch
NQ = 4           # SWDGE queues


def _cdiv(a, b):
    return -(-a // b)


# ----------------------------------------------------------------------------
# Host-side graph preprocessing (index manipulation only).
# ----------------------------------------------------------------------------
def _preprocess(x, edge_index):
    x = np.asarray(x, np.float32)
    ei = np.asarray(edge_index, np.int64)
    N, C = x.shape
    E = ei.shape[1]
    src, dst = ei[0], ei[1]

    shard = _cdiv(_cdiv(N, N_CORES), 128) * 128
    NP = shard * N_CORES
    NWIN = _cdiv(shard, WIN)

    deg = np.bincount(dst, minlength=N).astype(np.int64)
    recip_full = (1.0 / np.maximum(deg, 1)).astype(np.float32)

    # Degree-balanced snake deal over (core, 128-batch) bins.
    NBT = shard // 128
    nbins = N_CORES * NBT
    order = np.argsort(-deg, kind="stable")
    i = np.arange(N)
    r = i // nbins
    p = i % nbins
    binidx = np.where(r % 2 == 0, p, nbins - 1 - p)
    core_b = binidx % N_CORES
    bat_b = binidx // N_CORES
    newid = core_b * shard + bat_b * 128 + r
    perm = np.empty(N, np.int64)
    perm[order] = newid

    psrc = perm[src]
    pdst = perm[dst]
    LOTAB = min(32768, NP)

    x_tab = np.zeros((NP, C), np.float32)
    x_tab[perm] = x
    recip_bc_perm = np.zeros(NP, np.float32)
    recip_bc_perm[perm] = recip_full

    core_of = pdst // shard
    local = pdst % shard
    is_hi = (psrc >= LOTAB).astype(np.int64)

    # per-(core, half, local-dst) degree
    keyd = (core_of * 2 + is_hi) * shard + local
    degs = np.bincount(keyd, minlength=N_CORES * 2 * shard)
    degs = degs.reshape(N_CORES, 2, shard)             # [core, half, local]
    assert degs.max() <= 128, "single dst-half degree exceeds one tile"

    # Structural tile plan, uniform across cores: per (window, half), greedily
    # split the window's dsts into ranges where every core's segment sum fits
    # 128 slots. Tiles: (half, win, a, w); a is window-relative.
    tiles = []
    win_tiles = []        # per window: list of tile ids (lo first, then hi)
    for b in range(NWIN):
        wlo = b * WIN
        wsz = min(WIN, shard - wlo)
        tl = []
        for half in (0, 1):
            d = degs[:, half, wlo:wlo + wsz]           # [core, wsz]
            if half == 1 and d.sum() == 0:
                continue
            csum = np.concatenate(
                [np.zeros((N_CORES, 1), np.int64), np.cumsum(d, axis=1)],
                axis=1)
            a = 0
            while a < wsz:
                base = csum[:, a]
                w = 1
                while a + w < wsz and ((csum[:, a + w + 1] - base) <= 128).all():
                    w += 1
                tl.append((half, b, a, w))
                a += w
        win_tiles.append(list(range(len(tiles), len(tiles) + len(tl))))
        tiles.extend(tl)

    # stream ids per half (gather streams are per-half)
    lo_sid = {}
    hi_sid = {}
    for t, (half, b, a, w) in enumerate(tiles):
        if half == 0:
            lo_sid[t] = len(lo_sid)
        else:
            hi_sid[t] = len(hi_sid)
    TLO, THI = len(lo_sid), len(hi_sid)

    # --- per-core slot/S content -------------------------------------------
    keye = (core_of * 2 + is_hi) * shard + local
    ordr = np.argsort(keye, kind="stable")
    psrc_s = psrc[ordr]
    keye_s = keye[ordr]
    starts = np.concatenate([[0], np.cumsum(degs.reshape(-1))])
    rank = np.arange(E) - starts[keye_s]
    core_e = keye_s // (2 * shard)
    half_e = (keye_s // shard) % 2
    loc_e = keye_s % shard

    def wrap_idx(a_):
        return np.ascontiguousarray(
            np.tile(a_.reshape(-1, 16).T, (8, 1)).astype(np.int16))

    bf = ml_dtypes.bfloat16
    per_core = []
    for k in range(N_CORES):
        idxlo = np.zeros(TLO * 128, np.int32)
        idxhi = np.zeros(max(THI, 1) * 128, np.int32)
        S2 = np.zeros((2, 128, shard), np.float32)
        slot_base = np.zeros((2, shard), np.int64)
        for t, (half, b, a, w) in enumerate(tiles):
            sid = lo_sid[t] if half == 0 else hi_sid[t]
            dloc = b * WIN + a
            dsl = degs[k, half, dloc:dloc + w]
            offs = np.concatenate([[0], np.cumsum(dsl)])
            assert offs[-1] <= 128
            slot_base[half, dloc:dloc + w] = sid * 128 + offs[:-1]
            for j in range(w):
                if dsl[j]:
                    S2[half, offs[j]:offs[j + 1], dloc + j] = \
                        recip_bc_perm[k * shard + dloc + j]
        m = core_e == k
        slot = slot_base[half_e[m], loc_e[m]] + rank[m]
        vlo = half_e[m] == 0
        idxlo[slot[vlo]] = psrc_s[m][vlo]
        if THI:
            idxhi[slot[~vlo]] = psrc_s[m][~vlo] - LOTAB

        ent = {
            "idx_lo": wrap_idx(idxlo),
            "S_lo": S2[0].astype(bf),
            "xT_sh": np.ascontiguousarray(
                x_tab.T[:, k * shard:(k + 1) * shard]).astype(bf),
        }
        if THI:
            ent["idx_hi"] = wrap_idx(idxhi)
            ent["S_hi"] = S2[1].astype(bf)
        per_core.append(ent)

    meta = dict(NP=NP, shard=shard, NWIN=NWIN, C=C, LOTAB=LOTAB,
                TLO=TLO, THI=THI, tiles=tiles, win_tiles=win_tiles,
                lo_sid=lo_sid, hi_sid=hi_sid)
    return x_tab.astype(bf), per_core, perm, meta


# ----------------------------------------------------------------------------
# Bass program builder (one static SPMD program for all 8 cores).
# ----------------------------------------------------------------------------
def _build(meta, HID, OC):
    NP, shard, NWIN, C = meta["NP"], meta["shard"], meta["NWIN"], meta["C"]
    LOTAB, TLO, THI = meta["LOTAB"], meta["TLO"], meta["THI"]
    tiles = meta["tiles"]
    win_tiles = meta["win_tiles"]
    lo_sid, hi_sid = meta["lo_sid"], meta["hi_sid"]

    nc = bacc.Bacc("TRN2", target_bir_lowering=False, debug=False,
                   num_devices=N_CORES, num_swdge_queues=NQ)

    x_tab_d = nc.dram_tensor("x_tab", [NP, C], BF16, kind="ExternalInput")
    xT_sh_d = nc.dram_tensor("xT_sh", [C, shard], BF16, kind="ExternalInput")
    idxlo_d = nc.dram_tensor("idx_lo", [128, TLO * 8], I16, kind="ExternalInput")
    idxhi_d = (nc.dram_tensor("idx_hi", [128, THI * 8], I16,
                              kind="ExternalInput") if THI else None)
    slo_d = nc.dram_tensor("S_lo", [128, shard], BF16, kind="ExternalInput")
    shi_d = (nc.dram_tensor("S_hi", [128, shard], BF16, kind="ExternalInput")
             if THI else None)
    ident_d = nc.dram_tensor("ident", [128, 128], BF16, kind="ExternalInput")
    w1l_d = nc.dram_tensor("W1_l", [C, HID], BF16, kind="ExternalInput")
    w1r_d = nc.dram_tensor("W1_r", [C, HID], BF16, kind="ExternalInput")
    w2l_d = nc.dram_tensor("W2_l", [HID, OC], BF16, kind="ExternalInput")
    w2r_d = nc.dram_tensor("W2_r", [HID, OC], BF16, kind="ExternalInput")
    b1_d = nc.dram_tensor("b1", [HID, 1], F32, kind="ExternalInput")
    b2_d = nc.dram_tensor("b2", [OC, 1], F32, kind="ExternalInput")
    out_d = nc.dram_tensor("out", [shard, OC], F32, kind="ExternalOutput")

    NB = _cdiv(shard, NBATCH)

    with tile.TileContext(nc) as tc:
        with (
            tc.tile_pool(name="res", bufs=1) as rp,
            tc.tile_pool(name="mlo", bufs=16) as mlo_p,
            tc.tile_pool(name="mhi", bufs=16) as mhi_p,
            tc.tile_pool(name="hstage", bufs=3) as hsp,
            tc.tile_pool(name="aggp", bufs=3, space="PSUM") as aggp,
            tc.tile_pool(name="densep", bufs=1, space="PSUM") as densep,
            tc.tile_pool(name="tpp", bufs=1, space="PSUM") as tpp,
            tc.tile_pool(name="dramh", bufs=1, space="DRAM") as dram_h,
            tc.tile_pool(name="drama", bufs=1, space="DRAM") as dram_a,
        ):
            def load(shape, dtype, dram_t, name):
                t = rp.tile(shape, dtype, name=name, tag=name)
                nc.sync.dma_start(t[:], dram_t.ap())
                return t

            idxlo_sb = load([128, TLO * 8], I16, idxlo_d, "idxlo_sb")
            idxhi_sb = (load([128, THI * 8], I16, idxhi_d, "idxhi_sb")
                        if THI else None)
            xT_sb = load([C, shard], BF16, xT_sh_d, "xT_sb")
            slo_sb = load([128, shard], BF16, slo_d, "slo_sb")
            shi_sb = load([128, shard], BF16, shi_d, "shi_sb") if THI else None
            ident_sb = load([128, 128], BF16, ident_d, "ident_sb")
            w1l_sb = load([C, HID], BF16, w1l_d, "w1l_sb")
            w1r_sb = load([C, HID], BF16, w1r_d, "w1r_sb")
            w2l_sb = load([HID, OC], BF16, w2l_d, "w2l_sb")
            w2r_sb = load([HID, OC], BF16, w2r_d, "w2r_sb")
            b1_sb = load([HID, 1], F32, b1_d, "b1_sb")
            b2_sb = load([OC, 1], F32, b2_d, "b2_sb")

            h_full = dram_h.tile([NP, C], BF16, name="h_full",
                                 addr_space="Shared")
            h_loc = dram_a.tile([NP, C], BF16, name="h_loc")
            ag_in = dram_a.tile([shard, C], BF16, name="ag_in")

            hT_sb = rp.tile([128, shard], BF16, name="hT", tag="hT")

            def do_layer(layer):
                table = x_tab_d.ap() if layer == 0 else h_loc[:]
                agg_sb = rp.tile([128, shard], BF16, name=f"agg{layer}",
                                 tag="agg")

                chunk_lo, chunk_hi = {}, {}
                # interleave lo/hi calls by the window their first tile feeds
                sid_win = {0: {}, 1: {}}
                for t, (half, b, a, w) in enumerate(tiles):
                    sid = lo_sid[t] if half == 0 else hi_sid[t]
                    sid_win[half][sid] = b
                calls = []
                for half, Ttot in ((0, TLO), (1, THI)):
                    for c0 in range(0, Ttot, CT):
                        calls.append((sid_win[half][c0], half, c0,
                                      min(CT, Ttot - c0)))
                calls.sort()
                qn = 0
                for _, half, c0, nt in calls:
                    if half == 0:
                        tab = table[0:LOTAB, :]
                        idx_sb, mp, chunks = idxlo_sb, mlo_p, chunk_lo
                    else:
                        tab = table[LOTAB:NP, :]
                        idx_sb, mp, chunks = idxhi_sb, mhi_p, chunk_hi
                    m = mp.tile([128, CT, C], BF16, name=f"m{half}",
                                tag=f"m{half}")
                    nc.gpsimd.dma_gather(
                        out_ap=m[:, :nt, :],
                        in_ap=tab,
                        idxs_ap=idx_sb[:, c0 * 8:(c0 + nt) * 8],
                        num_idxs=nt * 128,
                        num_idxs_reg=nt * 128,
                        elem_size=C,
                        queue_num=qn % NQ,
                    )
                    qn += 1
                    chunks[c0 // CT] = m

                # ---- aggregation windows ----
                for b in range(NWIN):
                    wlo = b * WIN
                    wsz = min(WIN, shard - wlo)
                    tl = win_tiles[b]
                    has_hi = any(tiles[t][0] == 1 for t in tl)
                    psum = aggp.tile([128, WIN], F32, name="psum", tag="psum")
                    psum_hi = (aggp.tile([128, WIN], F32, name="psum_hi",
                                         tag="psum_hi") if has_hi else None)
                    for t in tl:
                        half, _, a, w = tiles[t]
                        if half == 0:
                            sid = lo_sid[t]
                            mt = chunk_lo[sid // CT][:, sid % CT, :]
                            S_sb = slo_sb
                            dst_ps = psum
                        else:
                            sid = hi_sid[t]
                            mt = chunk_hi[sid // CT][:, sid % CT, :]
                            S_sb = shi_sb
                            dst_ps = psum_hi
                        nc.tensor.matmul(
                            dst_ps[:, a:a + w], mt,
                            S_sb[:, wlo + a:wlo + a + w],
                            start=True, stop=True)
                    if has_hi:
                        hi_sb = hsp.tile([128, WIN], F32, name="hi_sb",
                                         tag="hi_sb")
                        nc.scalar.activation(
                            hi_sb[:, :wsz], psum_hi[:, :wsz],
                            mybir.ActivationFunctionType.Copy)
                        nc.vector.tensor_tensor(
                            agg_sb[:, wlo:wlo + wsz], psum[:, :wsz],
                            hi_sb[:, :wsz], mybir.AluOpType.add)
                    else:
                        nc.vector.tensor_copy(agg_sb[:, wlo:wlo + wsz],
                                              psum[:, :wsz])

                    # ---- dense for this window (WIN == NBATCH) ----
                    off = wlo
                    w = wsz
                    dp = densep.tile([128, NBATCH], F32, name="dp", tag="dp")
                    if layer == 0:
                        nc.tensor.matmul(dp[:HID, :w], w1l_sb[:],
                                         agg_sb[:, off:off + w],
                                         start=True, stop=False)
                        nc.tensor.matmul(dp[:HID, :w], w1r_sb[:],
                                         xT_sb[:, off:off + w],
                                         start=False, stop=True)
                        nc.scalar.activation(
                            hT_sb[:HID, off:off + w], dp[:HID, :w],
                            mybir.ActivationFunctionType.Relu, bias=b1_sb[:])
                        for s in range(0, w, 128):
                            wn = min(128, w - s)
                            tp = tpp.tile([128, 128], BF16, name="tp",
                                          tag="tp")
                            nc.tensor.transpose(
                                tp[:wn, :HID],
                                hT_sb[:HID, off + s:off + s + wn],
                                ident_sb[:HID, :HID])
                            hs = hsp.tile([128, C], BF16, name="hs", tag="hs")
                            nc.vector.tensor_copy(hs[:wn, :], tp[:wn, :HID])
                            nc.sync.dma_start(
                                ag_in[off + s:off + s + wn, :], hs[:wn, :])
                    else:
                        nc.tensor.matmul(dp[:OC, :w], w2l_sb[:],
                                         agg_sb[:, off:off + w],
                                         start=True, stop=False)
                        nc.tensor.matmul(dp[:OC, :w], w2r_sb[:],
                                         hT_sb[:HID, off:off + w],
                                         start=False, stop=True)
                        ot = hsp.tile([128, NBATCH], BF16, name="ot", tag="ot")
                        nc.scalar.activation(
                            ot[:OC, :w], dp[:OC, :w],
                            mybir.ActivationFunctionType.Identity,
                            bias=b2_sb[:])
                        for s in range(0, w, 128):
                            wn = min(128, w - s)
                            tp = tpp.tile([128, 128], BF16, name="tp",
                                          tag="tp")
                            nc.tensor.transpose(tp[:wn, :OC],
                                                ot[:OC, s:s + wn],
                                                ident_sb[:OC, :OC])
                            os_ = hsp.tile([128, OC], F32, name="os", tag="os")
                            nc.vector.tensor_copy(os_[:wn, :], tp[:wn, :OC])
                            nc.sync.dma_start(
                                out_d.ap()[off + s:off + s + wn, :],
                                os_[:wn, :])

            do_layer(0)
            if os.environ.get("K3_CC_DIRECT", "0") == "1":
                nc.gpsimd.collective_compute(
                    "AllGather", mybir.AluOpType.bypass,
                    replica_groups=[list(range(N_CORES))],
                    ins=[ag_in.opt()], outs=[h_loc.opt()])
            else:
                nc.gpsimd.collective_compute(
                    "AllGather", mybir.AluOpType.bypass,
                    replica_groups=[list(range(N_CORES))],
                    ins=[ag_in.opt()], outs=[h_full.opt()])
                # Shared-space reads are slow for the gather engine; copy the
                # gathered table into local DRAM (lo half split across two
                # queues so the lo gather stream starts sooner).
                engs = [nc.sync, nc.scalar, nc.gpsimd]
                for lo, hi in ((0, LOTAB), (LOTAB, NP)):
                    if hi <= lo:
                        continue
                    n3 = -(-(hi - lo) // 3)
                    for j, e in enumerate(engs):
                        a = lo + j * n3
                        b = min(lo + (j + 1) * n3, hi)
                        if b > a:
                            e.dma_start(h_loc[a:b, :], h_full[a:b, :])
            do_layer(1)

    nc.compile()
    return nc


_CACHE = {}


def kernel(x, edge_index, W1_l, b1, W1_r, W2_l, b2, W2_r):
    x = np.asarray(x, np.float32)
    HID = W1_l.shape[1]
    OC = W2_l.shape[1]
    N = x.shape[0]

    x_tab, per_core, perm, meta = _preprocess(x, edge_index)

    key = (meta["NP"], meta["TLO"], meta["THI"], tuple(meta["tiles"]),
           HID, OC)
    if key not in _CACHE:
        _CACHE[key] = _build(meta, HID, OC)
    nc = _CACHE[key]

    bf = ml_dtypes.bfloat16
    shared = {
        "x_tab": x_tab, "ident": np.eye(128, dtype=bf),
        "W1_l": np.asarray(W1_l, np.float32).astype(bf),
        "W1_r": np.asarray(W1_r, np.float32).astype(bf),
        "W2_l": np.asarray(W2_l, np.float32).astype(bf),
        "W2_r": np.asarray(W2_r, np.float32).astype(bf),
        "b1": np.asarray(b1, np.float32).reshape(HID, 1).copy(),
        "b2": np.asarray(b2, np.float32).reshape(OC, 1).copy(),
    }
    in_maps = []
    for k in range(N_CORES):
        m = dict(shared)
        m.update(per_core[k])
        in_maps.append(m)

    res = run_bass_kernel_spmd(nc, in_maps, core_ids=list(range(N_CORES)))
    out_full = np.concatenate([res.results[k]["out"] for k in range(N_CORES)],
                              axis=0)
    return np.ascontiguousarray(out_full[perm[:N]].astype(np.float32))
